# revision 5
# baseline (speedup 1.0000x reference)
"""LIF (leaky integrate-and-fire) forward recurrence on 8 Trainium2 NeuronCores.

Input  x: (T=16, B=128, N=16384) float32, time-major.
    m[t] = tau * v[t-1] + x[t]
    y[t] = (m[t] >= v_th)            spike, as 0.0/1.0
    v[t] = m[t] * (1 - y[t])         hard reset

Sharding: N split 8 ways (2048 per core); per-neuron recurrence, no
cross-core communication.  Host re-lays each shard as (B, T, N).

Engine split (vs the all-DVE baseline at 88us, which was bound by
32 serial fp32 tensor-tensor DVE ops ~2.29us each):
  - PE (tensor engine, otherwise idle) computes m = xh + xr + tau*v as
    three accumulating identity matmuls per 512-col PSUM bank, all at
    1 cycle/row: xh is fp16, xr is fp8e5m2, v is float32r.
  - ACT: sig = Sign(1 - m) from PSUM -> int8 (the output; host maps
    spike = sig <= 0).
  - DVE: v = (sig > 0) * m -> fp32r SBUF (single PSUM operand; walrus
    forbids two PSUM reads in one op, so the reset mask comes from sig).
Input is compressed host-side to 3 B/elem: x = fp16(x) + e5m2 residual
(exact to ~2^-14); v carries ~12 mantissa bits through fp32r.  Measured
end-to-end l2 error vs the f32 reference is ~5e-3 (a few hundred spike
flips out of 33.5M), well inside the 2e-2 gate; DMA drops from 21MB to
16.8MB per core (~40us floor at ~420GB/s/core).
"""

import numpy as np
import ml_dtypes

import concourse.bass as bass
import concourse.mybir as mybir
from concourse.bass_utils import run_bass_kernel_spmd
from concourse.mybir import AluOpType
from concourse.tile import TileContext

T, B, N = 16, 128, 16384
NCORES = 8
NSH = N // NCORES  # 2048 neurons per core
NB = NSH // 512  # PSUM banks per timestep tile
TAU = 0.5
V_TH = 1.0

IN_CHUNKS = [1, 1, 2, 4, 4, 4]
OUT_CHUNKS = [4, 4, 4, 2, 1, 1]

_cached_nc = None


def _split_multiwaits(nc):
    """Walrus codegen supports only ONE sync-wait per instruction; Tile
    sometimes attaches more.  Move extras onto same-engine NoOps."""
    multi_ok = (mybir.InstEventSemaphore, mybir.InstNoOp)
    for f in nc.m.functions:
        for b in f.blocks:
            new_insts = []
            for inst in b.instructions:
                si = inst.sync_info
                if (
                    not isinstance(inst, multi_ok)
                    and si is not None
                    and len(si.on_wait) > 1
                ):
                    waits = list(si.on_wait)
                    for j, w in enumerate(waits[:-1]):
                        new_insts.append(
                            mybir.InstNoOp(
                                name=f"{inst.name}_presync{j}",
                                engine=inst.engine,
                                sync_info=mybir.SyncInfo(on_wait=[w], on_update=[]),
                            )
                        )
                    inst.sync_info = mybir.SyncInfo(
                        on_wait=[waits[-1]], on_update=list(si.on_update)
                    )
                new_insts.append(inst)
            b.instructions = new_insts


def _build():
    nc = bass.Bass(trn_type="TRN2")
    xh = nc.dram_tensor("xh", [B, T, NSH], mybir.dt.float16, kind="ExternalInput")
    xr = nc.dram_tensor("xr", [B, T, NSH], mybir.dt.float8e5, kind="ExternalInput")
    wI = nc.dram_tensor("wI", [B, B], mybir.dt.float16, kind="ExternalInput")
    wR = nc.dram_tensor("wR", [B, B], mybir.dt.float8e5, kind="ExternalInput")
    wT = nc.dram_tensor("wT", [B, B], mybir.dt.float32r, kind="ExternalInput")
    sig = nc.dram_tensor("y", [B, T, NSH], mybir.dt.int8, kind="ExternalOutput")

    H = NSH // 2

    with TileContext(nc) as tc:
        with (
            tc.tile_pool(name="sb", bufs=1) as sb,
            tc.psum_pool(name="ps", bufs=2) as ps,
        ):
            xhs = sb.tile([B, T, NSH], mybir.dt.float16)
            xrs = sb.tile([B, T, NSH], mybir.dt.float8e5)
            wIs = sb.tile([B, B], mybir.dt.float16)
            wRs = sb.tile([B, B], mybir.dt.float8e5)
            wTs = sb.tile([B, B], mybir.dt.float32r)
            v = sb.tile([B, NSH], mybir.dt.float32r)
            sg = sb.tile([B, T, NSH], mybir.dt.int8)

            # weights ride the scalar (output) ring: it is idle at the
            # start, so they never delay the first x chunk on sync
            nc.scalar.dma_start(out=wIs[:], in_=wI[:])
            nc.scalar.dma_start(out=wRs[:], in_=wR[:])
            nc.scalar.dma_start(out=wTs[:], in_=wT[:])

            # input stream on the sync ring, fine ramp first
            t0 = 0
            for w in IN_CHUNKS:
                nc.sync.dma_start(out=xhs[:, t0 : t0 + w, :], in_=xh[:, t0 : t0 + w, :])
                nc.sync.dma_start(out=xrs[:, t0 : t0 + w, :], in_=xr[:, t0 : t0 + w, :])
                t0 += w

            mt = {}
            mt[0] = ps.tile([B, NSH], mybir.dt.float32, tag="m", name="m0")

            # PE warmup: dummy self-contained matmuls on the weight tile
            # during the initial input-DMA window, so the HAM clock gate
            # un-throttles (1.2 -> 2.4 GHz) before the real chain starts.
            # They scribble on m0's first bank; the real x-matmul resets it
            # (start=True clears has_written).
            for _ in range(24):
                nc.tensor.matmul(
                    out=mt[0][:, :B], lhsT=wIs[:], rhs=wIs[:],
                    start=True, stop=True, skip_group_check=True,
                )

            def x_mms(t):
                if t not in mt:
                    mt[t] = ps.tile([B, NSH], mybir.dt.float32, tag="m", name=f"m{t}")
                m = mt[t]
                for b in range(NB):
                    cs = slice(b * 512, (b + 1) * 512)
                    nc.tensor.matmul(
                        out=m[:, cs], lhsT=wIs[:], rhs=xhs[:, t, cs],
                        start=True, stop=False,
                    )
                    nc.tensor.matmul(
                        out=m[:, cs], lhsT=wRs[:], rhs=xrs[:, t, cs],
                        start=False, stop=(t == 0),
                    )

            # chunked output stores, emitted right after the sig that
            # completes each chunk (scalar ring is FIFO: emitting late
            # would serialize the whole output stream after the last sig)
            out_edges = {}
            t0 = 0
            for w in OUT_CHUNKS:
                if t0 + w < T:  # final step handled per-bank below
                    out_edges[t0 + w - 1] = (t0, w)
                t0 += w

            # The serial chain is v_mm(t,b) -> sig(t,b) -> vop(t,b) ->
            # v_mm(t+1,b), pipelined at PSUM-bank (512 col) granularity:
            # chain latency per bank (~1.9us) hides under the step period
            # while each engine serves the four bank-chains round-robin.
            # x-matmuls for t+1 are emitted after v_mms(t): they refill
            # the PE during the sig/vop latency of step t.
            x_mms(0)
            for t in range(T):
                m = mt[t]
                for b in range(NB):
                    cs = slice(b * 512, (b + 1) * 512)
                    if t > 0:
                        nc.tensor.matmul(
                            out=m[:, cs], lhsT=wTs[:], rhs=v[:, cs],
                            start=False, stop=True,
                        )
                    nc.scalar.activation(
                        sg[:, t, cs], m[:, cs],
                        mybir.ActivationFunctionType.Sign,
                        bias=V_TH, scale=-1.0,
                    )
                    if t < T - 1:
                        nc.vector.scalar_tensor_tensor(
                            v[:, cs], sg[:, t, cs], 0, m[:, cs],
                            AluOpType.is_gt, AluOpType.mult,
                        )
                    else:
                        # final step: store each bank as soon as its sig
                        # lands so the output drain overlaps
                        nc.scalar.dma_start(
                            out=sig[:, t : t + 1, cs], in_=sg[:, t : t + 1, cs]
                        )
                if t + 1 < T:
                    x_mms(t + 1)
                if t in out_edges:
                    t0, w = out_edges[t]
                    nc.scalar.dma_start(
                        out=sig[:, t0 : t0 + w, :], in_=sg[:, t0 : t0 + w, :]
                    )
    _split_multiwaits(nc)
    return nc


def kernel(x: np.ndarray) -> np.ndarray:
    global _cached_nc
    if _cached_nc is None:
        _cached_nc = _build()
    nc = _cached_nc

    x = np.ascontiguousarray(x, dtype=np.float32)
    assert x.shape == (T, B, N)
    # (T, B, N) -> per-core (B, T, NSH) shards; split x = fp16 + e5m2 residual
    xbt = np.ascontiguousarray(x.transpose(1, 0, 2))
    xh = xbt.astype(np.float16)
    xr = (xbt - xh.astype(np.float32)).astype(ml_dtypes.float8_e5m2)
    wI = np.eye(B, dtype=np.float16)
    wR = np.eye(B, dtype=ml_dtypes.float8_e5m2)
    wT = (TAU * np.eye(B)).astype(np.float32)
    in_maps = [
        {
            "xh": np.ascontiguousarray(xh[:, :, k * NSH : (k + 1) * NSH]),
            "xr": np.ascontiguousarray(xr[:, :, k * NSH : (k + 1) * NSH]),
            "wI": wI,
            "wR": wR,
            "wT": wT,
        }
        for k in range(NCORES)
    ]
    res = run_bass_kernel_spmd(nc, in_maps, core_ids=list(range(NCORES)))
    global _last_exec_ns
    if res.exec_time_ns is not None:
        _last_exec_ns = res.exec_time_ns
    # per-core int8 sign (B, T, NSH): sig = Sign(1-m), spike <=> sig <= 0
    out = np.concatenate([r["y"] for r in res.results], axis=2)
    return (
        np.ascontiguousarray(out.transpose(1, 0, 2)) <= 0
    ).astype(np.float32)


_last_exec_ns = None


# revision 9
# speedup vs baseline: 1.3836x; 1.3836x over previous
"""LIF (leaky integrate-and-fire) forward recurrence on 8 Trainium2 NeuronCores.

Input  x: (T=16, B=128, N=16384) float32, time-major.
    m[t] = tau * v[t-1] + x[t]
    y[t] = (m[t] >= v_th)            spike, as 0.0/1.0
    v[t] = m[t] * (1 - y[t])         hard reset

Sharding: N split 8 ways (2048 per core); per-neuron recurrence, no
cross-core communication.  Host re-lays each shard as (B, T, N).

Engine split (vs the all-DVE baseline at 88us, which was bound by
32 serial fp32 tensor-tensor DVE ops ~2.29us each):
  - PE (tensor engine, otherwise idle) computes m = xh + xr + tau*v as
    three accumulating identity matmuls per 512-col PSUM bank, all at
    1 cycle/row: xh is fp16, xr is fp8e5m2, v is float32r.
  - ACT: sig = Sign(1 - m) from PSUM -> int8 (the output; host maps
    spike = sig <= 0).
  - DVE: v = (sig > 0) * m -> fp32r SBUF (single PSUM operand; walrus
    forbids two PSUM reads in one op, so the reset mask comes from sig).
Input is compressed host-side to 3 B/elem: x = fp16(x) + e5m2 residual
(exact to ~2^-14); v carries ~12 mantissa bits through fp32r.  Measured
end-to-end l2 error vs the f32 reference is ~5e-3 (a few hundred spike
flips out of 33.5M), well inside the 2e-2 gate; DMA drops from 21MB to
16.8MB per core (~40us floor at ~420GB/s/core).
"""

import numpy as np
import ml_dtypes

import concourse.bass as bass
import concourse.mybir as mybir
from concourse.bass_utils import run_bass_kernel_spmd
from concourse.mybir import AluOpType
from concourse.tile import TileContext

T, B, N = 16, 128, 16384
NCORES = 8
NSH = N // NCORES  # 2048 neurons per core
NB = NSH // 512  # PSUM banks per timestep tile
TAU = 0.5
V_TH = 1.0

IN_CHUNKS = [1, 1, 2, 4, 4, 4]
OUT_CHUNKS = [4, 4, 4, 2, 1, 1]

_cached_nc = None


def _split_multiwaits(nc):
    """Walrus codegen supports only ONE sync-wait per instruction; Tile
    sometimes attaches more.  Move extras onto same-engine NoOps."""
    multi_ok = (mybir.InstEventSemaphore, mybir.InstNoOp)
    for f in nc.m.functions:
        for b in f.blocks:
            new_insts = []
            for inst in b.instructions:
                si = inst.sync_info
                if (
                    not isinstance(inst, multi_ok)
                    and si is not None
                    and len(si.on_wait) > 1
                ):
                    waits = list(si.on_wait)
                    for j, w in enumerate(waits[:-1]):
                        new_insts.append(
                            mybir.InstNoOp(
                                name=f"{inst.name}_presync{j}",
                                engine=inst.engine,
                                sync_info=mybir.SyncInfo(on_wait=[w], on_update=[]),
                            )
                        )
                    inst.sync_info = mybir.SyncInfo(
                        on_wait=[waits[-1]], on_update=list(si.on_update)
                    )
                new_insts.append(inst)
            b.instructions = new_insts


def _build():
    nc = bass.Bass(trn_type="TRN2")
    xh = nc.dram_tensor("xh", [B, T, NSH], mybir.dt.float16, kind="ExternalInput")
    xr = nc.dram_tensor("xr", [B, T, NSH], mybir.dt.float8e5, kind="ExternalInput")
    wI = nc.dram_tensor("wI", [B, B], mybir.dt.float16, kind="ExternalInput")
    wR = nc.dram_tensor("wR", [B, B], mybir.dt.float8e5, kind="ExternalInput")
    wT = nc.dram_tensor("wT", [B, B], mybir.dt.float32r, kind="ExternalInput")
    sig = nc.dram_tensor("y", [B, T, NSH], mybir.dt.int8, kind="ExternalOutput")

    H = NSH // 2

    with TileContext(nc) as tc:
        with (
            tc.tile_pool(name="sb", bufs=1) as sb,
            tc.tile_pool(name="sgp", bufs=2) as sgp,
            tc.psum_pool(name="ps", bufs=2) as ps,
        ):
            xhs = sb.tile([B, T, NSH], mybir.dt.float16)
            xrs = sb.tile([B, T, NSH], mybir.dt.float8e5)
            wIs = sb.tile([B, B], mybir.dt.float16)
            wRs = sb.tile([B, B], mybir.dt.float8e5)
            wTs = sb.tile([B, B], mybir.dt.float32r)
            # per-bank state tiles: dependency tracking is per-TILE, so
            # each 512-col bank-chain gets its own tiles to keep the four
            # chains independent (one shared tile serializes all banks)
            vb = [sb.tile([B, 512], mybir.dt.float32r, name=f"v{b}") for b in range(NB)]

            # weights ride the scalar (output) ring: it is idle at the
            # start, so they never delay the first x chunk on sync
            nc.scalar.dma_start(out=wIs[:], in_=wI[:])
            nc.scalar.dma_start(out=wRs[:], in_=wR[:])
            nc.scalar.dma_start(out=wTs[:], in_=wT[:])

            # input stream on the sync ring, fine ramp first
            t0 = 0
            for w in IN_CHUNKS:
                nc.sync.dma_start(out=xhs[:, t0 : t0 + w, :], in_=xh[:, t0 : t0 + w, :])
                nc.sync.dma_start(out=xrs[:, t0 : t0 + w, :], in_=xr[:, t0 : t0 + w, :])
                t0 += w

            # output chunking: (start, width) per chunk, and for each t the
            # chunk it belongs to
            chunks = []
            t0 = 0
            for w in OUT_CHUNKS:
                chunks.append((t0, w))
                t0 += w
            chunk_of = {}
            for ci, (t0, w) in enumerate(chunks):
                for t in range(t0, t0 + w):
                    chunk_of[t] = ci

            # per-bank PSUM m tiles (4 tags x 2 bufs = all 8 banks)
            mt = {}  # (t, b) -> psum tile

            def m_tile(t, b):
                if (t, b) not in mt:
                    mt[(t, b)] = ps.tile(
                        [B, 512], mybir.dt.float32, tag=f"m{b}", name=f"m{t}_{b}"
                    )
                return mt[(t, b)]

            # per-bank, per-chunk sg tiles (double-buffered so the chunk
            # store never WAR-blocks the next chunk's sig writes)
            sgt = {}  # (ci, b) -> sbuf int8 tile [B, w, 512]

            def sg_tile(t, b):
                ci = chunk_of[t]
                if (ci, b) not in sgt:
                    sgt[(ci, b)] = sgp.tile(
                        [B, 4, 512], mybir.dt.int8, tag=f"sg{b}", name=f"sg{ci}_{b}"
                    )
                return sgt[(ci, b)]

            # PE warmup: dummy self-contained matmuls on the weight tile
            # during the initial input-DMA window, so the HAM clock gate
            # un-throttles (1.2 -> 2.4 GHz) before the real chain starts.
            # They scribble on m(0,0); the real x-matmul resets it
            # (start=True clears has_written).
            for _ in range(24):
                nc.tensor.matmul(
                    out=m_tile(0, 0)[:, :B], lhsT=wIs[:], rhs=wIs[:],
                    start=True, stop=True, skip_group_check=True,
                )

            def x_mms(t):
                for b in range(NB):
                    cs = slice(b * 512, (b + 1) * 512)
                    nc.tensor.matmul(
                        out=m_tile(t, b)[:], lhsT=wIs[:], rhs=xhs[:, t, cs],
                        start=True, stop=False,
                    )
                for b in range(NB):
                    cs = slice(b * 512, (b + 1) * 512)
                    nc.tensor.matmul(
                        out=m_tile(t, b)[:], lhsT=wRs[:], rhs=xrs[:, t, cs],
                        start=False, stop=(t == 0),
                    )

            # The serial chain per bank b is v_mm(t,b) -> sig(t,b) ->
            # vop(t,b) -> v_mm(t+1,b); the four bank-chains are fully
            # independent (separate m/v/sg tiles) and pipeline round-robin
            # across PE/ACT/DVE.  x-matmuls for t+1 are emitted after
            # v_mms(t): they refill the PE during the sig/vop latency.
            x_mms(0)
            for t in range(T):
                ci = chunk_of[t]
                c0, cw = chunks[ci]
                for b in range(NB):
                    cs = slice(b * 512, (b + 1) * 512)
                    if t > 0:
                        nc.tensor.matmul(
                            out=m_tile(t, b)[:], lhsT=wTs[:], rhs=vb[b][:],
                            start=False, stop=True,
                        )
                    nc.scalar.activation(
                        sg_tile(t, b)[:, t - c0, :], m_tile(t, b)[:],
                        mybir.ActivationFunctionType.Sign,
                        bias=V_TH, scale=-1.0,
                    )
                    if t < T - 1:
                        nc.vector.scalar_tensor_tensor(
                            vb[b][:], sg_tile(t, b)[:, t - c0, :], 0, m_tile(t, b)[:],
                            AluOpType.is_gt, AluOpType.mult,
                        )
                if t + 1 < T:
                    x_mms(t + 1)
                if t == c0 + cw - 1:
                    # chunk finished: store each bank's sg tile
                    for b in range(NB):
                        cs = slice(b * 512, (b + 1) * 512)
                        nc.scalar.dma_start(
                            out=sig[:, c0 : c0 + cw, cs], in_=sgt[(ci, b)][:, :cw, :]
                        )
    _split_multiwaits(nc)
    return nc


def kernel(x: np.ndarray) -> np.ndarray:
    global _cached_nc
    if _cached_nc is None:
        _cached_nc = _build()
    nc = _cached_nc

    x = np.ascontiguousarray(x, dtype=np.float32)
    assert x.shape == (T, B, N)
    # (T, B, N) -> per-core (B, T, NSH) shards; split x = fp16 + e5m2 residual
    xbt = np.ascontiguousarray(x.transpose(1, 0, 2))
    xh = xbt.astype(np.float16)
    xr = (xbt - xh.astype(np.float32)).astype(ml_dtypes.float8_e5m2)
    wI = np.eye(B, dtype=np.float16)
    wR = np.eye(B, dtype=ml_dtypes.float8_e5m2)
    wT = (TAU * np.eye(B)).astype(np.float32)
    in_maps = [
        {
            "xh": np.ascontiguousarray(xh[:, :, k * NSH : (k + 1) * NSH]),
            "xr": np.ascontiguousarray(xr[:, :, k * NSH : (k + 1) * NSH]),
            "wI": wI,
            "wR": wR,
            "wT": wT,
        }
        for k in range(NCORES)
    ]
    res = run_bass_kernel_spmd(nc, in_maps, core_ids=list(range(NCORES)))
    global _last_exec_ns
    if res.exec_time_ns is not None:
        _last_exec_ns = res.exec_time_ns
    # per-core int8 sign (B, T, NSH): sig = Sign(1-m), spike <=> sig <= 0
    out = np.concatenate([r["y"] for r in res.results], axis=2)
    return (
        np.ascontiguousarray(out.transpose(1, 0, 2)) <= 0
    ).astype(np.float32)


_last_exec_ns = None


# revision 13
# speedup vs baseline: 1.4050x; 1.0155x over previous
"""LIF (leaky integrate-and-fire) forward recurrence on 8 Trainium2 NeuronCores.

Input  x: (T=16, B=128, N=16384) float32, time-major.
    m[t] = tau * v[t-1] + x[t]
    y[t] = (m[t] >= v_th)            spike, as 0.0/1.0
    v[t] = m[t] * (1 - y[t])         hard reset

Sharding: N split 8 ways (2048 per core); per-neuron recurrence, no
cross-core communication.  Host re-lays each shard as (B, T, N).

Engine split (vs the all-DVE baseline at 88us, which was bound by
32 serial fp32 tensor-tensor DVE ops ~2.29us each):
  - PE (tensor engine, otherwise idle) computes m = xh + xr + tau*v as
    three accumulating identity matmuls per 512-col PSUM bank, all at
    1 cycle/row: xh is fp16, xr is fp8e5m2, v is float32r.
  - ACT: sig = Sign(1 - m) from PSUM -> int8 (the output; host maps
    spike = sig <= 0).
  - DVE: v = (sig > 0) * m -> fp32r SBUF (single PSUM operand; walrus
    forbids two PSUM reads in one op, so the reset mask comes from sig).
Input is compressed host-side to 3 B/elem: x = fp16(x) + e5m2 residual
(exact to ~2^-14); v carries ~12 mantissa bits through fp32r.  Measured
end-to-end l2 error vs the f32 reference is ~5e-3 (a few hundred spike
flips out of 33.5M), well inside the 2e-2 gate; DMA drops from 21MB to
16.8MB per core (~40us floor at ~420GB/s/core).
"""

import numpy as np
import ml_dtypes

import concourse.bass as bass
import concourse.mybir as mybir
from concourse.bass_utils import run_bass_kernel_spmd
from concourse.mybir import AluOpType
from concourse.tile import TileContext

T, B, N = 16, 128, 16384
NCORES = 8
NSH = N // NCORES  # 2048 neurons per core
NB = NSH // 512  # PSUM banks per timestep tile
TAU = 0.5
V_TH = 1.0

IN_CHUNKS = [1, 1, 2, 4, 4, 4]
OUT_CHUNKS = [4, 4, 4, 2, 1, 1]

_cached_nc = None


def _split_multiwaits(nc):
    """Walrus codegen supports only ONE sync-wait per instruction; Tile
    sometimes attaches more.  Move extras onto same-engine NoOps."""
    multi_ok = (mybir.InstEventSemaphore, mybir.InstNoOp)
    for f in nc.m.functions:
        for b in f.blocks:
            new_insts = []
            for inst in b.instructions:
                si = inst.sync_info
                if (
                    not isinstance(inst, multi_ok)
                    and si is not None
                    and len(si.on_wait) > 1
                ):
                    waits = list(si.on_wait)
                    for j, w in enumerate(waits[:-1]):
                        new_insts.append(
                            mybir.InstNoOp(
                                name=f"{inst.name}_presync{j}",
                                engine=inst.engine,
                                sync_info=mybir.SyncInfo(on_wait=[w], on_update=[]),
                            )
                        )
                    inst.sync_info = mybir.SyncInfo(
                        on_wait=[waits[-1]], on_update=list(si.on_update)
                    )
                new_insts.append(inst)
            b.instructions = new_insts


def _build():
    nc = bass.Bass(trn_type="TRN2")
    xh = nc.dram_tensor("xh", [B, T, NSH], mybir.dt.float16, kind="ExternalInput")
    xr = nc.dram_tensor("xr", [B, T, NSH], mybir.dt.float8e5, kind="ExternalInput")
    wI = nc.dram_tensor("wI", [B, B], mybir.dt.float16, kind="ExternalInput")
    wR = nc.dram_tensor("wR", [B, B], mybir.dt.float8e5, kind="ExternalInput")
    wT = nc.dram_tensor("wT", [B, B], mybir.dt.float32r, kind="ExternalInput")
    # bank-major output: each per-bank chunk store writes contiguous
    # (cw*512 B) runs per partition instead of 512-B strided fragments
    sig = nc.dram_tensor("y", [B, NB, T, 512], mybir.dt.int8, kind="ExternalOutput")

    H = NSH // 2

    with TileContext(nc) as tc:
        with (
            tc.tile_pool(name="sb", bufs=1) as sb,
            tc.tile_pool(name="sgp", bufs=2) as sgp,
            tc.psum_pool(name="ps", bufs=2) as ps,
        ):
            xhs = sb.tile([B, T, NSH], mybir.dt.float16)
            xrs = sb.tile([B, T, NSH], mybir.dt.float8e5)
            wIs = sb.tile([B, B], mybir.dt.float16)
            wRs = sb.tile([B, B], mybir.dt.float8e5)
            wTs = sb.tile([B, B], mybir.dt.float32r)
            # per-bank state tiles: dependency tracking is per-TILE, so
            # each 512-col bank-chain gets its own tiles to keep the four
            # chains independent (one shared tile serializes all banks)
            vb = [sb.tile([B, 512], mybir.dt.float32r, name=f"v{b}") for b in range(NB)]

            # weights go FIRST on the sync ring (in-order per ring): they
            # gate the PE warmup and every matmul, and rings do not
            # interleave fairly enough to trust the other ring early
            nc.sync.dma_start(out=wIs[:], in_=wI[:])
            nc.sync.dma_start(out=wRs[:], in_=wR[:])
            nc.sync.dma_start(out=wTs[:], in_=wT[:])

            # input stream on the sync ring, fine ramp first
            t0 = 0
            for w in IN_CHUNKS:
                nc.sync.dma_start(out=xhs[:, t0 : t0 + w, :], in_=xh[:, t0 : t0 + w, :])
                nc.sync.dma_start(out=xrs[:, t0 : t0 + w, :], in_=xr[:, t0 : t0 + w, :])
                t0 += w

            # output chunking: (start, width) per chunk, and for each t the
            # chunk it belongs to
            chunks = []
            t0 = 0
            for w in OUT_CHUNKS:
                chunks.append((t0, w))
                t0 += w
            chunk_of = {}
            for ci, (t0, w) in enumerate(chunks):
                for t in range(t0, t0 + w):
                    chunk_of[t] = ci

            # per-bank PSUM m tiles (4 tags x 2 bufs = all 8 banks)
            mt = {}  # (t, b) -> psum tile

            def m_tile(t, b):
                if (t, b) not in mt:
                    mt[(t, b)] = ps.tile(
                        [B, 512], mybir.dt.float32, tag=f"m{b}", name=f"m{t}_{b}"
                    )
                return mt[(t, b)]

            # per-bank, per-chunk sg tiles (double-buffered so the chunk
            # store never WAR-blocks the next chunk's sig writes)
            sgt = {}  # (ci, b) -> sbuf int8 tile [B, w, 512]

            def sg_tile(t, b):
                ci = chunk_of[t]
                if (ci, b) not in sgt:
                    sgt[(ci, b)] = sgp.tile(
                        [B, 4, 512], mybir.dt.int8, tag=f"sg{b}", name=f"sg{ci}_{b}"
                    )
                return sgt[(ci, b)]

            # PE warmup: dummy self-contained matmuls on the weight tile
            # during the initial input-DMA window, so the HAM clock gate
            # un-throttles (1.2 -> 2.4 GHz) before the real chain starts.
            # They scribble on m(0,0); the real x-matmul resets it
            # (start=True clears has_written).
            for _ in range(24):
                nc.tensor.matmul(
                    out=m_tile(0, 0)[:, :B], lhsT=wIs[:], rhs=wIs[:],
                    start=True, stop=True, skip_group_check=True,
                )

            def x_mms(t):
                for b in range(NB):
                    cs = slice(b * 512, (b + 1) * 512)
                    nc.tensor.matmul(
                        out=m_tile(t, b)[:], lhsT=wIs[:], rhs=xhs[:, t, cs],
                        start=True, stop=False,
                    )
                for b in range(NB):
                    cs = slice(b * 512, (b + 1) * 512)
                    nc.tensor.matmul(
                        out=m_tile(t, b)[:], lhsT=wRs[:], rhs=xrs[:, t, cs],
                        start=False, stop=(t == 0),
                    )

            # The serial chain per bank b is v_mm(t,b) -> sig(t,b) ->
            # vop(t,b) -> v_mm(t+1,b); the four bank-chains are fully
            # independent (separate m/v/sg tiles) and pipeline round-robin
            # across PE/ACT/DVE.  x-matmuls for t+1 are emitted after
            # v_mms(t): they refill the PE during the sig/vop latency.
            x_mms(0)
            for t in range(T):
                ci = chunk_of[t]
                c0, cw = chunks[ci]
                for b in range(NB):
                    cs = slice(b * 512, (b + 1) * 512)
                    if t > 0:
                        nc.tensor.matmul(
                            out=m_tile(t, b)[:], lhsT=wTs[:], rhs=vb[b][:],
                            start=False, stop=True,
                        )
                    nc.scalar.activation(
                        sg_tile(t, b)[:, t - c0, :], m_tile(t, b)[:],
                        mybir.ActivationFunctionType.Sign,
                        bias=V_TH, scale=-1.0,
                    )
                    if t < T - 1:
                        nc.vector.scalar_tensor_tensor(
                            vb[b][:], sg_tile(t, b)[:, t - c0, :], 0, m_tile(t, b)[:],
                            AluOpType.is_gt, AluOpType.mult,
                        )
                if t + 1 < T:
                    x_mms(t + 1)
                if t == c0 + cw - 1:
                    # chunk finished: store each bank's sg tile (scalar
                    # ring; contiguous runs thanks to bank-major layout)
                    for b in range(NB):
                        nc.scalar.dma_start(
                            out=sig[:, b, c0 : c0 + cw, :], in_=sgt[(ci, b)][:, :cw, :]
                        )
    _split_multiwaits(nc)
    return nc


def kernel(x: np.ndarray) -> np.ndarray:
    global _cached_nc
    if _cached_nc is None:
        _cached_nc = _build()
    nc = _cached_nc

    x = np.ascontiguousarray(x, dtype=np.float32)
    assert x.shape == (T, B, N)
    # (T, B, N) -> per-core (B, T, NSH) shards; split x = fp16 + e5m2 residual
    xbt = np.ascontiguousarray(x.transpose(1, 0, 2))
    xh = xbt.astype(np.float16)
    xr = (xbt - xh.astype(np.float32)).astype(ml_dtypes.float8_e5m2)
    wI = np.eye(B, dtype=np.float16)
    wR = np.eye(B, dtype=ml_dtypes.float8_e5m2)
    wT = (TAU * np.eye(B)).astype(np.float32)
    in_maps = [
        {
            "xh": np.ascontiguousarray(xh[:, :, k * NSH : (k + 1) * NSH]),
            "xr": np.ascontiguousarray(xr[:, :, k * NSH : (k + 1) * NSH]),
            "wI": wI,
            "wR": wR,
            "wT": wT,
        }
        for k in range(NCORES)
    ]
    res = run_bass_kernel_spmd(nc, in_maps, core_ids=list(range(NCORES)))
    global _last_exec_ns
    if res.exec_time_ns is not None:
        _last_exec_ns = res.exec_time_ns
    # per-core int8 sign, bank-major (B, NB, T, 512): sig = Sign(1-m),
    # spike <=> sig <= 0.  Un-permute banks then cores then time-major.
    outs = [
        r["y"].transpose(0, 2, 1, 3).reshape(B, T, NSH) for r in res.results
    ]
    out = np.concatenate(outs, axis=2)
    return (
        np.ascontiguousarray(out.transpose(1, 0, 2)) <= 0
    ).astype(np.float32)


_last_exec_ns = None


# revision 16
# speedup vs baseline: 1.5871x; 1.1296x over previous
"""LIF (leaky integrate-and-fire) forward recurrence on 8 Trainium2 NeuronCores.

Input  x: (T=16, B=128, N=16384) float32, time-major.
    m[t] = tau * v[t-1] + x[t]
    y[t] = (m[t] >= v_th)            spike, as 0.0/1.0
    v[t] = m[t] * (1 - y[t])         hard reset

Sharding: N split 8 ways (2048 per core); per-neuron recurrence, no
cross-core communication.  Host re-lays each shard as (B, T, N).

Engine split (vs the all-DVE baseline at 88us, which was bound by
32 serial fp32 tensor-tensor DVE ops ~2.29us each):
  - PE (tensor engine, otherwise idle) computes m = xh + xr + tau*v as
    three accumulating identity matmuls per 512-col PSUM bank, all at
    1 cycle/row: xh is fp16, xr is fp8e5m2, v is float32r.
  - ACT: sig = Sign(1 - m) from PSUM -> int8 (the output; host maps
    spike = sig <= 0).
  - DVE: v = (sig > 0) * m -> fp32r SBUF (single PSUM operand; walrus
    forbids two PSUM reads in one op, so the reset mask comes from sig).
Input is compressed host-side to 3 B/elem: x = fp16(x) + e5m2 residual
(exact to ~2^-14); v carries ~12 mantissa bits through fp32r.  Measured
end-to-end l2 error vs the f32 reference is ~5e-3 (a few hundred spike
flips out of 33.5M), well inside the 2e-2 gate; DMA drops from 21MB to
16.8MB per core (~40us floor at ~420GB/s/core).
"""

import numpy as np
import ml_dtypes

import concourse.bass as bass
import concourse.mybir as mybir
from concourse.bass_utils import run_bass_kernel_spmd
from concourse.mybir import AluOpType
from concourse.tile import TileContext

T, B, N = 16, 128, 16384
NCORES = 8
NSH = N // NCORES  # 2048 neurons per core
NB = NSH // 512  # PSUM banks per timestep tile
TAU = 0.5
V_TH = 1.0

IN_CHUNKS = [1, 1, 2, 4, 4, 4]
OUT_CHUNKS = [4, 4, 4, 2, 1, 1]

_cached_nc = None


def _split_multiwaits(nc):
    """Walrus codegen supports only ONE sync-wait per instruction; Tile
    sometimes attaches more.  Move extras onto same-engine NoOps."""
    multi_ok = (mybir.InstEventSemaphore, mybir.InstNoOp)
    for f in nc.m.functions:
        for b in f.blocks:
            new_insts = []
            for inst in b.instructions:
                si = inst.sync_info
                if (
                    not isinstance(inst, multi_ok)
                    and si is not None
                    and len(si.on_wait) > 1
                ):
                    waits = list(si.on_wait)
                    for j, w in enumerate(waits[:-1]):
                        new_insts.append(
                            mybir.InstNoOp(
                                name=f"{inst.name}_presync{j}",
                                engine=inst.engine,
                                sync_info=mybir.SyncInfo(on_wait=[w], on_update=[]),
                            )
                        )
                    inst.sync_info = mybir.SyncInfo(
                        on_wait=[waits[-1]], on_update=list(si.on_update)
                    )
                new_insts.append(inst)
            b.instructions = new_insts


def _build():
    nc = bass.Bass(trn_type="TRN2")
    xh = nc.dram_tensor("xh", [B, T, NSH], mybir.dt.float16, kind="ExternalInput")
    xr = nc.dram_tensor("xr", [B, T, NSH], mybir.dt.float8e5, kind="ExternalInput")
    wI = nc.dram_tensor("wI", [B, B], mybir.dt.float16, kind="ExternalInput")
    wR = nc.dram_tensor("wR", [B, B], mybir.dt.float8e5, kind="ExternalInput")
    wT = nc.dram_tensor("wT", [B, B], mybir.dt.float32r, kind="ExternalInput")
    # bank-major output: each per-bank chunk store writes contiguous
    # (cw*512 B) runs per partition instead of 512-B strided fragments
    sig = nc.dram_tensor("y", [B, NB, T, 512], mybir.dt.int8, kind="ExternalOutput")

    H = NSH // 2

    with TileContext(nc) as tc:
        with (
            tc.tile_pool(name="sb", bufs=1) as sb,
            tc.tile_pool(name="sgp", bufs=2) as sgp,
            tc.psum_pool(name="ps", bufs=2) as ps,
        ):
            xhs = sb.tile([B, T, NSH], mybir.dt.float16)
            xrs = sb.tile([B, T, NSH], mybir.dt.float8e5)
            wIs = sb.tile([B, B], mybir.dt.float16)
            wRs = sb.tile([B, B], mybir.dt.float8e5)
            wTs = sb.tile([B, B], mybir.dt.float32r)
            # per-bank state tiles: dependency tracking is per-TILE, so
            # each 512-col bank-chain gets its own tiles to keep the four
            # chains independent (one shared tile serializes all banks)
            vb = [sb.tile([B, 512], mybir.dt.float32r, name=f"v{b}") for b in range(NB)]

            # weights go FIRST on the sync ring (in-order per ring): they
            # gate the PE warmup and every matmul, and rings do not
            # interleave fairly enough to trust the other ring early
            nc.sync.dma_start(out=wIs[:], in_=wI[:])
            nc.sync.dma_start(out=wRs[:], in_=wR[:])
            nc.sync.dma_start(out=wTs[:], in_=wT[:])

            # input stream on the sync ring, one xh/xr pair per timestep:
            # x_mms(t) then gate on step t's own data, not a chunk tail,
            # so compute rides the stream with no chunk-edge stalls
            for t in range(T):
                nc.sync.dma_start(out=xhs[:, t : t + 1, :], in_=xh[:, t : t + 1, :])
                nc.sync.dma_start(out=xrs[:, t : t + 1, :], in_=xr[:, t : t + 1, :])

            # output chunking: (start, width) per chunk, and for each t the
            # chunk it belongs to
            chunks = []
            t0 = 0
            for w in OUT_CHUNKS:
                chunks.append((t0, w))
                t0 += w
            chunk_of = {}
            for ci, (t0, w) in enumerate(chunks):
                for t in range(t0, t0 + w):
                    chunk_of[t] = ci

            # per-bank PSUM m tiles (4 tags x 2 bufs = all 8 banks)
            mt = {}  # (t, b) -> psum tile

            def m_tile(t, b):
                if (t, b) not in mt:
                    mt[(t, b)] = ps.tile(
                        [B, 512], mybir.dt.float32, tag=f"m{b}", name=f"m{t}_{b}"
                    )
                return mt[(t, b)]

            # per-bank, per-chunk sg tiles (double-buffered so the chunk
            # store never WAR-blocks the next chunk's sig writes)
            sgt = {}  # (ci, b) -> sbuf int8 tile [B, w, 512]

            def sg_tile(t, b):
                ci = chunk_of[t]
                if (ci, b) not in sgt:
                    sgt[(ci, b)] = sgp.tile(
                        [B, 4, 512], mybir.dt.int8, tag=f"sg{b}", name=f"sg{ci}_{b}"
                    )
                return sgt[(ci, b)]

            # PE warmup: dummy self-contained matmuls on a junk tile that
            # is never written, so they start immediately at t=0 (no DMA
            # wait) and the HAM clock gate un-throttles (1.2 -> 2.4 GHz)
            # before the real chain starts.  They scribble on m(0,0); the
            # real x-matmul resets it (start=True clears has_written).
            junk = sb.tile([B, B], mybir.dt.bfloat16, name="junk")
            nc.vector.memset(junk[:], 0.0)
            for _ in range(30):
                nc.tensor.matmul(
                    out=m_tile(0, 0)[:, :B], lhsT=junk[:], rhs=junk[:],
                    start=True, stop=True, skip_group_check=True,
                )

            def x_mms(t):
                for b in range(NB):
                    cs = slice(b * 512, (b + 1) * 512)
                    nc.tensor.matmul(
                        out=m_tile(t, b)[:], lhsT=wIs[:], rhs=xhs[:, t, cs],
                        start=True, stop=False,
                    )
                for b in range(NB):
                    cs = slice(b * 512, (b + 1) * 512)
                    nc.tensor.matmul(
                        out=m_tile(t, b)[:], lhsT=wRs[:], rhs=xrs[:, t, cs],
                        start=False, stop=(t == 0),
                    )

            # The serial chain per bank b is v_mm(t,b) -> sig(t,b) ->
            # vop(t,b) -> v_mm(t+1,b); the four bank-chains are fully
            # independent (separate m/v/sg tiles) and pipeline round-robin
            # across PE/ACT/DVE.  x-matmuls for t+1 are emitted after
            # v_mms(t): they refill the PE during the sig/vop latency.
            x_mms(0)
            for t in range(T):
                ci = chunk_of[t]
                c0, cw = chunks[ci]
                for b in range(NB):
                    cs = slice(b * 512, (b + 1) * 512)
                    if t > 0:
                        nc.tensor.matmul(
                            out=m_tile(t, b)[:], lhsT=wTs[:], rhs=vb[b][:],
                            start=False, stop=True,
                        )
                    nc.scalar.activation(
                        sg_tile(t, b)[:, t - c0, :], m_tile(t, b)[:],
                        mybir.ActivationFunctionType.Sign,
                        bias=V_TH, scale=-1.0,
                    )
                    if t < T - 1:
                        nc.vector.scalar_tensor_tensor(
                            vb[b][:], sg_tile(t, b)[:, t - c0, :], 0, m_tile(t, b)[:],
                            AluOpType.is_gt, AluOpType.mult,
                        )
                if t + 1 < T:
                    x_mms(t + 1)
                if t == c0 + cw - 1:
                    # chunk finished: store each bank's sg tile (scalar
                    # ring; contiguous runs thanks to bank-major layout)
                    for b in range(NB):
                        nc.scalar.dma_start(
                            out=sig[:, b, c0 : c0 + cw, :], in_=sgt[(ci, b)][:, :cw, :]
                        )
    _split_multiwaits(nc)
    return nc


def kernel(x: np.ndarray) -> np.ndarray:
    global _cached_nc
    if _cached_nc is None:
        _cached_nc = _build()
    nc = _cached_nc

    x = np.ascontiguousarray(x, dtype=np.float32)
    assert x.shape == (T, B, N)
    # (T, B, N) -> per-core (B, T, NSH) shards; split x = fp16 + e5m2 residual
    xbt = np.ascontiguousarray(x.transpose(1, 0, 2))
    xh = xbt.astype(np.float16)
    xr = (xbt - xh.astype(np.float32)).astype(ml_dtypes.float8_e5m2)
    wI = np.eye(B, dtype=np.float16)
    wR = np.eye(B, dtype=ml_dtypes.float8_e5m2)
    wT = (TAU * np.eye(B)).astype(np.float32)
    in_maps = [
        {
            "xh": np.ascontiguousarray(xh[:, :, k * NSH : (k + 1) * NSH]),
            "xr": np.ascontiguousarray(xr[:, :, k * NSH : (k + 1) * NSH]),
            "wI": wI,
            "wR": wR,
            "wT": wT,
        }
        for k in range(NCORES)
    ]
    res = run_bass_kernel_spmd(nc, in_maps, core_ids=list(range(NCORES)))
    global _last_exec_ns
    if res.exec_time_ns is not None:
        _last_exec_ns = res.exec_time_ns
    # per-core int8 sign, bank-major (B, NB, T, 512): sig = Sign(1-m),
    # spike <=> sig <= 0.  Un-permute banks then cores then time-major.
    outs = [
        r["y"].transpose(0, 2, 1, 3).reshape(B, T, NSH) for r in res.results
    ]
    out = np.concatenate(outs, axis=2)
    return (
        np.ascontiguousarray(out.transpose(1, 0, 2)) <= 0
    ).astype(np.float32)


_last_exec_ns = None
